# revision 32
# baseline (speedup 1.0000x reference)
"""CrossMerge kernel for Trainium2 (8 NeuronCores, data-parallel over batch).

Computation (per batch b):
    inv[k]  = stable argsort of vec_indices[b, :, k]              (k = 0, 1)
    s_k[u,d] = ys[b,k,d,u] + ys[b,k+2,d,L-1-u]   (fwd + flipped half, pre-summed)
    out[b,d,l] = sum_k s_k[inv[k][l], d]

Device plan per core (2 batches):
  Phase A (per b,k): load ys[b,k]/[b,k+2] in [d,l] layout, one full-width
           vector add with the second operand read in reversed l order,
           TensorE-transpose 128-wide l tiles into [l,d] rows, store to a
           per-(b,k) DRAM table (rows interleaved as r = (u%128)*25 + u//128
           so the store is one contiguous DMA).
  Phase B (per b,k): ONE dma_gather custom instruction pulls all 3136
           permuted 768B rows into SBUF (descriptor generation on the Q7
           CounterMachine; single_packet=False is required on this runtime).
  Phase C: TensorE-transpose gathered blocks back to [d,l] with the k-pair
           sum done by PSUM accumulation; store contiguous per d-half.

Host does only sharding + argsort-derived index prep (metadata for the DMA
descriptors); all tensor data movement/compute is on device.
"""
import sys

sys.path.insert(0, "/opt/trn_rl_repo")

import numpy as np

import concourse.bacc as bacc
import concourse.bass as bass
import concourse.mybir as mybir
import concourse.tile as tile
from concourse.bass_utils import run_bass_kernel_spmd
from concourse.masks import make_identity

# Problem constants (hardcoded per contract).
B, K, D, H, W = 16, 4, 192, 56, 56
L = H * W          # 3136
K2 = K // 2        # 2
NCORES = 8
BL = B // NCORES   # 2 batches per core
C = 25             # l tiles of 128: 24 full + 1 of 64
LP = C * 128       # 3200 padded
DH = 96            # d half (192 = 2*96)
NW = LP // 16      # idx columns in wrapped int16 layout (200)
F32 = mybir.dt.float32
I16 = mybir.dt.int16


def crossmerge_body(tc, out_ap, ys_ap, idx_ap):
    """Tile kernel body.

    out_ap: [BL, 192, 3136] f32 (ExternalOutput)
    ys_ap:  [BL, 4, 192, 3136] f32 (ExternalInput)
    idx_ap: [BL, 2, 128, 200] i16, wrapped dma_gather row indices
    """
    nc = tc.nc
    # per-(b,k) scratch tables -> fine-grained store->gather dependencies
    s_tabs = [
        [nc.dram_tensor(f"s_scratch_{b}_{k}", [LP, D], F32,
                        kind="Internal").ap() for k in range(K2)]
        for b in range(BL)
    ]

    with (
        tc.tile_pool(name="const", bufs=1) as cpool,
        tc.tile_pool(name="y", bufs=3) as ypool,
        tc.tile_pool(name="sum", bufs=2) as spool,
        tc.tile_pool(name="stage", bufs=2) as stpool,
        tc.tile_pool(name="gather", bufs=3) as gpool,
        tc.tile_pool(name="idx", bufs=4) as ipool,
        tc.tile_pool(name="ostage", bufs=2) as opool,
        tc.tile_pool(name="psA", bufs=4, space="PSUM") as psA,
        tc.tile_pool(name="psC", bufs=4, space="PSUM") as psC,
    ):
        ident = cpool.tile([128, 128], F32)
        make_identity(nc, ident[:])

        # idx tiles first in the sync queue — nothing blocks them, and the
        # phase-B gathers need them immediately
        idx_tiles = []
        for b in range(BL):
            i0 = ipool.tile([128, NW], I16, tag="idx")
            i1 = ipool.tile([128, NW], I16, tag="idx")
            nc.sync.dma_start(out=i0[:], in_=idx_ap[b, 0])
            nc.sync.dma_start(out=i1[:], in_=idx_ap[b, 1])
            idx_tiles.append((i0, i1))

        # ---- Phase A: build the four s tables ----
        for b in range(BL):
            for k in range(K2):
                stage = stpool.tile([128, C * D], F32, tag="stage")
                stage3 = stage[:].rearrange("p (c d) -> p c d", c=C)
                for h in range(2):
                    y1 = ypool.tile([DH, L], F32, tag="y")
                    y2 = ypool.tile([DH, L], F32, tag="y")
                    yv1 = ys_ap[b, k].rearrange("(a p) l -> p a l", p=DH)
                    yv2 = ys_ap[b, k + K2].rearrange("(a p) l -> p a l", p=DH)
                    nc.sync.dma_start(out=y1[:], in_=yv1[:, h, :])
                    nc.sync.dma_start(out=y2[:], in_=yv2[:, h, :])
                    # s[u] = y1[u] + y2[L-1-u], one full-width add
                    st = spool.tile([DH, L], F32, tag="sum")
                    nc.vector.tensor_add(
                        out=st[:], in0=y1[:], in1=y2[:, ::-1])
                    # transpose 128-wide l tiles; batch 5 per PSUM bank
                    # (c=24 is 64 rows — keep it in its own tile so every
                    # batched copy reads only fully-written PSUM)
                    for c0 in list(range(0, C - 1, 5)) + [C - 1]:
                        cn = min(5, C - 1 - c0) if c0 < C - 1 else 1
                        sz = 128 if c0 < C - 1 else L - (C - 1) * 128
                        ps = psA.tile([128, 5 * DH], F32, space="PSUM")
                        for j in range(cn):
                            c = c0 + j
                            nc.tensor.transpose(
                                out=ps[:sz, j * DH:(j + 1) * DH],
                                in_=st[:, c * 128:c * 128 + sz],
                                identity=ident[:DH, :DH],
                            )
                        nc.vector.tensor_copy(
                            out=stage3[:sz, c0:c0 + cn, h * DH:(h + 1) * DH],
                            in_=ps[:sz, :cn * DH].rearrange(
                                "p (j e) -> p j e", j=cn),
                        )
                blk = s_tabs[b][k][:].rearrange("(p c) d -> p c d", c=C)
                # stores go on the scalar engine's HWDGE queue so they never
                # head-of-line-block the load stream on the sync queue;
                # split at the c=15 copy-batch boundary so the first chunk
                # transfers while the last stage copies finish (the table-
                # complete sem gates gather descgen); pad rows (p>=64, c=24)
                # are never written, so they are skipped
                nc.scalar.dma_start(out=blk[:, :15, :],
                                    in_=stage3[:, :15, :])
                nc.scalar.dma_start(out=blk[:, 15:C - 1, :],
                                    in_=stage3[:, 15:C - 1, :])
                nc.scalar.dma_start(out=blk[:64, C - 1, :],
                                    in_=stage3[:64, C - 1, :])

        # ---- Phases B+C per b ----
        for b in range(BL):
            i0, i1 = idx_tiles[b]
            g0 = gpool.tile([128, C * D], F32, tag="g")
            g1 = gpool.tile([128, C * D], F32, tag="g")
            g0v = g0[:].rearrange("p (c d) -> p c d", c=C)
            g1v = g1[:].rearrange("p (c d) -> p c d", c=C)
            # custom gather instructions: row i of the permutation lands at
            # [i % 128, i // 128, :]; the int16 index list is 16-partition-
            # wrapped with trailing -1 padding. Each (b, k) gather is split
            # in two (earlier phase-C start) and k0/k1 go on separate SWDGE
            # queues so their latency-bound drains overlap.
            for k, (gv_, it) in ((0, (g0v, i0)), (1, (g1v, i1))):
                for quarter, (c_lo, c_hi) in enumerate(
                        ((0, 7), (7, 13), (13, 19), (19, C))):
                    n_idx = (c_hi - c_lo) * 128
                    n_valid = min(L, c_hi * 128) - c_lo * 128
                    # quarter-gathers spread over all 4 SWDGE queues: each
                    # drain starts right after its ~6us descgen chunk and
                    # phase-C batches unlock per quarter
                    nc.gpsimd.dma_gather(
                        out_ap=gv_[:, c_lo:c_hi, :],
                        in_ap=s_tabs[b][k][:],
                        idxs_ap=it[:, c_lo * 8:c_hi * 8],
                        num_idxs=n_idx, num_idxs_reg=n_valid, elem_size=D,
                        single_packet=False, queue_num=quarter)
            # transpose back to [d, l]; k-pair sum via PSUM accumulation;
            # batch 4 c-blocks per PSUM bank; store per d-half
            for h in range(2):
                ost = opool.tile([DH, L], F32, tag="ost")
                for c0 in range(0, C, 4):
                    cn = min(4, C - c0)
                    ps2 = psC.tile([DH, 512], F32, space="PSUM")
                    w = 0
                    for j in range(cn):
                        c = c0 + j
                        sz = min(128, L - c * 128)
                        nc.tensor.matmul(
                            out=ps2[:, w:w + sz],
                            lhsT=g0v[:sz, c, h * DH:(h + 1) * DH],
                            rhs=ident[:sz, :sz],
                            is_transpose=True, start=True, stop=False)
                        nc.tensor.matmul(
                            out=ps2[:, w:w + sz],
                            lhsT=g1v[:sz, c, h * DH:(h + 1) * DH],
                            rhs=ident[:sz, :sz],
                            is_transpose=True, start=False, stop=True)
                        w += sz
                    nc.vector.tensor_copy(
                        out=ost[:, c0 * 128:c0 * 128 + w], in_=ps2[:, :w])
                # split store: first half departs while the tail batches'
                # copies are still finishing
                ov = out_ap[b].rearrange("(a p) l -> p a l", p=DH)
                mid = 12 * 128
                nc.scalar.dma_start(out=ov[:, h, :mid], in_=ost[:, :mid])
                nc.scalar.dma_start(out=ov[:, h, mid:], in_=ost[:, mid:])


def _host_prep(ys, vec_indices):
    """Shard inputs and build gather index tensors."""
    ys = np.ascontiguousarray(np.asarray(ys, dtype=np.float32)).reshape(
        B, K, D, L)
    vi = np.asarray(vec_indices)
    inv = np.argsort(vi, axis=1, kind="stable")          # [B, L, K2]
    invt = np.transpose(inv, (0, 2, 1))                  # [B, K2, L]
    r = ((invt % 128) * C + (invt // 128)).astype(np.int16)
    # pad to 3200 with -1 (ignored by dma_gather), wrap in 16 partitions,
    # replicate to the 8 gpsimd core groups
    rpad = np.concatenate(
        [r, np.full((B, K2, LP - L), -1, dtype=np.int16)], axis=2)
    w = rpad.reshape(B, K2, NW, 16).transpose(0, 1, 3, 2)  # [B, K2, 16, NW]
    w = np.tile(w, (1, 1, 8, 1))                           # [B, K2, 128, NW]
    in_maps = []
    for i in range(NCORES):
        in_maps.append({
            "ys": ys[BL * i:BL * (i + 1)],
            "idx": np.ascontiguousarray(w[BL * i:BL * (i + 1)]),
        })
    return in_maps


_PROGRAM = None


def _build_program():
    global _PROGRAM
    if _PROGRAM is not None:
        return _PROGRAM
    nc = bacc.Bacc("TRN2", target_bir_lowering=False, debug=False,
                   enable_asserts=False, num_devices=NCORES,
                   num_swdge_queues=4)
    ys_t = nc.dram_tensor("ys", [BL, K, D, L], F32, kind="ExternalInput")
    idx_t = nc.dram_tensor("idx", [BL, K2, 128, NW], I16, kind="ExternalInput")
    out_t = nc.dram_tensor("out", [BL, D, L], F32, kind="ExternalOutput")
    with tile.TileContext(nc) as tc:
        crossmerge_body(tc, out_t.ap(), ys_t.ap(), idx_t.ap())
    nc.compile()
    _PROGRAM = nc
    return nc


def kernel(ys, vec_indices):
    nc = _build_program()
    in_maps = _host_prep(ys, vec_indices)
    res = run_bass_kernel_spmd(nc, in_maps, list(range(NCORES)))
    out = np.concatenate([r["out"] for r in res.results], axis=0)
    return out
